# revision 41
# baseline (speedup 1.0000x reference)
"""GATv2 3-layer GNN (EpitopeGNN) Bass/Trainium2 kernel, 8-core SPMD.

Design:
  - dst-node sharding, slot-aligned: nodes are degree-sorted and assigned
    (core = rank%8, block = within-core-rank//128, slot = rank%128) so every
    SBUF partition p of a block holds ONLY edges whose destination is the
    node at slot p.  Tiles-per-block T[b] = max node degree in that block
    (degree-quantile blocks keep padding tight); one static SPMD program.
  - Per layer: fl = h@Wlf, fr = h@Wrf per block on PE (att magnitudes/signs
    and the lrelu 0.6 factor folded into the weights, dims permuted so
    att>=0 dims come first); fl is AllGathered (2 pieces, overlapping the
    previous layer's edge phase); fr stays core-local.
  - Edge phase per 2-block group: one dma_gather of fl[src] rows (bf16
    256B rows); fr added via a broadcast AP (direct 32KB block load, no
    gather); logit = reduce(u) in ONE full-width DVE reduce where
    u = t +- (2/3)|t| (Abs on ACT, sign baked into the tables); pad slots
    masked with -30 before exp.  Softmax+aggregation in one pass:
    numerator accumulates diag(exp(logit)) @ FL on PE in PSUM, denominator
    is a row-reduce of exp(logit); un-fold via 1/m; LayerNorm; ReLU; PE
    transpose into per-block hT tiles; next layer's transforms chain per
    block (overlap with the current edge phase).
  - No segment-max needed: logits are bounded (few units) at these scales,
    and the dst-side linear term cancels in the softmax.
"""

import os
import sys

import numpy as np
import ml_dtypes

if "/opt/trn_rl_repo" not in sys.path:
    sys.path.insert(0, "/opt/trn_rl_repo")

SLOPE = 0.2
LN_EPS = 1e-5


def _cdiv(a, b):
    return -(-a // b)


class Cfg:
    def __init__(self, N=20000, E=320000, D_IN=1280, D=128, L=3, NC=8):
        assert D == 128 and D_IN % 128 == 0
        self.N, self.E, self.D_IN, self.D, self.L, self.NC = N, E, D_IN, D, L, NC
        self.NPC = _cdiv(N, NC)                      # real nodes per core
        self.NPC_PAD = _cdiv(self.NPC, 128) * 128    # padded nodes per core
        self.NB = self.NPC_PAD // 128                # dst blocks per core
        self.NPAD = NC * self.NPC_PAD
        self.KC = D_IN // 128                        # k-chunks for W_in


def _prep_edges(cfg, edge_index):
    """Degree-balanced node->(core,block,slot) assignment + per-core gather
    index / dst-local arrays + shared per-block tile counts.

    Returns (T, offs, EPAD, src_g, fr_g, dl_g, node_perm) where node_perm[n]
    is the padded global slot of original node n (out = out_dev[node_perm]).
    """
    N, NC, NPC, NPC_PAD, NB = cfg.N, cfg.NC, cfg.NPC, cfg.NPC_PAD, cfg.NB
    loops = np.arange(N, dtype=np.int64)
    src = np.concatenate([edge_index[0].astype(np.int64), loops])
    dst = np.concatenate([edge_index[1].astype(np.int64), loops])

    # Slot-aligned layout: partition p of block b holds ONLY edges whose dst
    # is the node at slot p (so dst_local == partition everywhere: fr comes
    # from a direct per-block load, the softmax one-hot is diagonal, and the
    # denominator is a row reduce).  Nodes are degree-sorted: core = rank%NC
    # (load balance), block = within-core-rank//128 (similar degrees share a
    # block so T[b]=max-degree stays tight).
    deg = np.bincount(dst, minlength=N)
    order = np.argsort(-deg, kind="stable")           # rank -> node
    core_of_node = np.empty(N, dtype=np.int64)
    blk_of_node = np.empty(N, dtype=np.int64)
    slot_of = np.empty(N, dtype=np.int64)
    rank = np.arange(N)
    core_of_node[order] = rank % NC
    within = rank // NC                               # degree rank within core
    blk_of_node[order] = within // 128
    slot_of[order] = within % 128
    node_perm = core_of_node * NPC_PAD + blk_of_node * 128 + slot_of
    # fl_full is all-gathered in NH pieces: rows ordered
    # [piece, core, local % (NPC_PAD/NH)] — gather ids must match that layout.
    import os as _os
    NH = int(_os.environ.get("GATV2_HALVES", "2"))
    H2 = NPC_PAD // NH
    local = blk_of_node * 128 + slot_of
    gat_id = (local // H2) * NC * H2 + core_of_node * H2 + (local % H2)

    # shared tiles-per-block = max degree of any node in that block (over cores)
    T = np.ones(NB, dtype=np.int64)
    degb = np.zeros((NC, NB), dtype=np.int64)
    np.maximum.at(degb, (core_of_node, blk_of_node), deg)
    T = np.maximum(1, degb.max(axis=0))
    EPAD = int(T.sum()) * 128
    offs = np.concatenate([[0], np.cumsum(T[:-1])]) * 128

    psrc = gat_id[src]
    dcore = core_of_node[dst]
    dblk = blk_of_node[dst]
    dslot = slot_of[dst]

    src_g, fr_g, dl_g = [], [], []
    for c in range(NC):
        sidx = np.zeros(EPAD, dtype=np.int16)
        msk = np.full(EPAD, -30.0, dtype=np.float32)
        sel = np.nonzero(dcore == c)[0]
        # per-dst running tile index
        key = dblk[sel] * 128 + dslot[sel]
        o = np.argsort(key, kind="stable")
        sel = sel[o]
        key = key[o]
        # position within each dst group = running count
        tpos = np.arange(len(sel)) - np.searchsorted(key, key, side="left")
        slot = offs[dblk[sel]] + tpos * 128 + dslot[sel]
        sidx[slot] = psrc[sel].astype(np.int16)
        msk[slot] = 0.0
        src_g.append(np.tile(sidx.reshape(-1, 16).T, (8, 1)).copy())
        fr_g.append(None)
        # mask layout: element i -> [i%128, i//128]
        dl_g.append(msk.reshape(-1, 128).T.copy())
    return T, offs, EPAD, src_g, fr_g, dl_g, node_perm


def _prep_weights(cfg, W_in, b_in, Wl, bl, Wr, br, att, conv_bias, ln_w, ln_b,
                  W_out, b_out):
    """Sign-sort attention dims per layer, fold |att| into Wl/Wr, chain perms."""
    L, D = cfg.L, cfg.D
    bf16 = ml_dtypes.bfloat16
    out = {}
    perm_prev = np.arange(D)
    layers = []
    for l in range(L):
        a = np.asarray(att[l, 0], dtype=np.float64)
        order = np.argsort(a < 0, kind="stable")     # att>=0 dims first
        K = int((a >= 0).sum())
        # 0.6 factor of the lrelu linear part folded into the tables:
        #   logit = sum_d u_d,  u = t' +- (2/3)|t'|  with t' = s*0.6*m*(xl+xr)
        # where s=+1 for d<K and s=-1 for d>=K (sign baked into the tables so
        # a single full-width reduce of u gives the logit directly).
        m = np.maximum(np.abs(a[order]), 1e-8) * 0.6
        sgn = np.where(np.arange(D) < K, 1.0, -1.0)
        ms = m * sgn
        wlf = (np.asarray(Wl[l], np.float64)[:, order] * ms[None, :])[perm_prev, :]
        wrf = (np.asarray(Wr[l], np.float64)[:, order] * ms[None, :])[perm_prev, :]
        blf = np.asarray(bl[l], np.float64)[order] * ms
        brf = np.asarray(br[l], np.float64)[order] * ms
        layers.append(dict(
            K=K, m=(ms).astype(np.float32),
            wlf=wlf.astype(bf16), wrf=wrf.astype(bf16),
            blf=blf.astype(np.float32), brf=brf.astype(np.float32),
            cb=np.asarray(conv_bias[l], np.float32)[order],
            lnw=np.asarray(ln_w[l], np.float32)[order],
            lnb=np.asarray(ln_b[l], np.float32)[order],
        ))
        perm_prev = order
    out["layers"] = layers
    out["w_in"] = np.asarray(W_in, np.float64).reshape(cfg.KC, 128, D) \
        .transpose(1, 0, 2).reshape(128, cfg.KC * D).astype(bf16)
    out["b_in"] = np.asarray(b_in, np.float32)
    out["w_out"] = np.asarray(W_out, np.float64)[perm_prev, :].astype(bf16)
    out["b_out"] = float(np.asarray(b_out).reshape(-1)[0])
    return out


def _build_program(cfg, wp, T, offs, EPAD):
    import concourse.bacc as bacc
    import concourse.bass as bass
    import concourse.mybir as mybir
    import concourse.tile as tile

    dt = mybir.dt
    Alu = mybir.AluOpType
    Act = mybir.ActivationFunctionType
    D, L, NB, KC = cfg.D, cfg.L, cfg.NB, cfg.KC
    NPC_PAD, NPAD, NC = cfg.NPC_PAD, cfg.NPAD, cfg.NC

    lay = wp["layers"]
    has_b_in = bool(np.any(wp["b_in"]))
    has_blf = [bool(np.any(l["blf"])) for l in lay]
    has_brf = [bool(np.any(l["brf"])) for l in lay]
    has_cb = [bool(np.any(l["cb"])) for l in lay]
    has_lnw = [not np.allclose(l["lnw"], 1.0) for l in lay]
    has_lnb = [bool(np.any(l["lnb"])) for l in lay]
    has_b_out = wp["b_out"] != 0.0

    nc = bacc.Bacc("TRN2", target_bir_lowering=False, debug=False,
                   enable_asserts=False, num_devices=NC)

    # ---- I/O ----
    xT = nc.dram_tensor("xT", [NB, 128, KC * D], dt.bfloat16, kind="ExternalInput")
    src_gidx = nc.dram_tensor("src_gidx", [128, EPAD // 16], dt.int16, kind="ExternalInput")
    mask_d = nc.dram_tensor("mask", [128, EPAD // 128], dt.float32, kind="ExternalInput")
    w_in_d = nc.dram_tensor("w_in", [128, KC * D], dt.bfloat16, kind="ExternalInput")
    wlf_d = nc.dram_tensor("wlf", [L, 128, D], dt.bfloat16, kind="ExternalInput")
    wrf_d = nc.dram_tensor("wrf", [L, 128, D], dt.bfloat16, kind="ExternalInput")
    invm_d = nc.dram_tensor("invm", [L, 128, D], dt.float32, kind="ExternalInput")
    wout_d = nc.dram_tensor("wout", [128, 1], dt.bfloat16, kind="ExternalInput")
    ident_d = nc.dram_tensor("ident", [128, 128], dt.bfloat16, kind="ExternalInput")
    aux_d = nc.dram_tensor("aux", [L, 6, 128, D], dt.float32, kind="ExternalInput") \
        if any(has_cb) or any(has_lnw) or any(has_lnb) or has_b_in or any(has_blf) or any(has_brf) \
        else None
    out_d = nc.dram_tensor("out", [128, NB], dt.float32, kind="ExternalOutput")

    # ---- internal DRAM ----
    fl_own = [nc.dram_tensor(f"fl_own{l}", [NPC_PAD, D], dt.bfloat16) for l in range(L)]
    fl_full = [nc.dram_tensor(f"fl_full{l}", [NPAD, D], dt.bfloat16, addr_space="Shared")
               for l in range(L)]
    fr_loc = [nc.dram_tensor(f"fr_loc{l}", [NPC_PAD, D], dt.bfloat16) for l in range(L)]

    with tile.TileContext(nc) as tc:
        with (
            tc.tile_pool(name="const", bufs=1) as cpool,
            tc.tile_pool(name="hT", bufs=1) as hpool,
            tc.tile_pool(name="xb", bufs=2) as xpool,
            tc.tile_pool(name="gatherG", bufs=3) as gpoolG,
            tc.tile_pool(name="gatherC", bufs=2) as gpoolC,
            tc.tile_pool(name="edge", bufs=4) as epool,
            tc.tile_pool(name="ep", bufs=2) as eppool,
            tc.tile_pool(name="psA", bufs=2, space="PSUM") as psA,
            tc.tile_pool(name="psB", bufs=2, space="PSUM") as psB,
            tc.tile_pool(name="psC", bufs=2, space="PSUM") as psC,
            tc.tile_pool(name="psT", bufs=2, space="PSUM") as psT,
        ):
            # ---- constants into SBUF ----
            ident_sb = cpool.tile([128, 128], dt.bfloat16, tag="ident")
            nc.sync.dma_start(ident_sb[:, :], ident_d[:, :])
            w_in_sb = cpool.tile([128, KC * D], dt.bfloat16, tag="w_in")
            nc.sync.dma_start(w_in_sb[:, :], w_in_d[:, :])
            wlf_sb = cpool.tile([128, L * D], dt.bfloat16, tag="wlf")
            wrf_sb = cpool.tile([128, L * D], dt.bfloat16, tag="wrf")
            invm_sb = cpool.tile([128, L * D], dt.float32, tag="invm")
            for l in range(L):
                nc.sync.dma_start(wlf_sb[:, l * D:(l + 1) * D], wlf_d[l, :, :])
                nc.sync.dma_start(wrf_sb[:, l * D:(l + 1) * D], wrf_d[l, :, :])
                nc.sync.dma_start(invm_sb[:, l * D:(l + 1) * D], invm_d[l, :, :])
            wout_sb = cpool.tile([128, 1], dt.bfloat16, tag="wout")
            nc.sync.dma_start(wout_sb[:, :], wout_d[:, :])
            eps_sb = cpool.tile([128, 1], dt.float32, tag="eps")
            nc.vector.memset(eps_sb[:, :], LN_EPS)
            eps30_sb = cpool.tile([128, 1], dt.float32, tag="eps30")
            nc.vector.memset(eps30_sb[:, :], 1e-30)
            sidx_all = cpool.tile([128, EPAD // 16], dt.int16, tag="sidx_all")
            nc.sync.dma_start(sidx_all[:, :], src_gidx[:, :])
            mask_all = cpool.tile([128, EPAD // 128], dt.float32, tag="mask_all")
            nc.sync.dma_start(mask_all[:, :], mask_d[:, :])
            aux_sb = None
            if aux_d is not None:
                aux_sb = cpool.tile([128, L * 6 * D], dt.float32, tag="aux")
                for l in range(L):
                    for j in range(6):
                        nc.sync.dma_start(
                            aux_sb[:, (l * 6 + j) * D:(l * 6 + j + 1) * D],
                            aux_d[l, j, :, :])

            def aux_ap(l, j):
                return aux_sb[:, (l * 6 + j) * D:(l * 6 + j + 1) * D]

            hTt = [hpool.tile([128, 128], dt.bfloat16, tag=f"hT{b}",
                              name=f"hT{b}") for b in range(NB)]
            out_sb = cpool.tile([128, NB], dt.float32, tag="out_sb")

            def make_hT(h_sb, b):
                """h_sb [128n,128d] bf16 -> hT[:, b*128:(b+1)*128] (transposed)."""
                pT = psT.tile([128, 128], dt.bfloat16, tag="pT")
                nc.tensor.transpose(pT[:, :], h_sb[:, :], ident_sb[:, :])
                nc.scalar.copy(hTt[b][:, :], pT[:, :])

            def _epilogue(l, b, pnum, pden):
                """normalize by denom, unfold 1/m, (+conv_bias), LN, relu, hT."""
                den = eppool.tile([128, 1], dt.float32, tag="den")
                nc.vector.tensor_scalar(den[:, :], pden[:, :], 1e-30, None, Alu.add)
                rden = eppool.tile([128, 1], dt.float32, tag="rden")
                nc.vector.reciprocal(rden[:, :], den[:, :])
                y = eppool.tile([128, 128], dt.float32, tag="y")
                nc.scalar.activation(y[:, :], pnum[:, :], Act.Copy, scale=rden[:, :])
                z = eppool.tile([128, 128], dt.float32, tag="z")
                s0 = eppool.tile([128, 1], dt.float32, tag="s0")
                nc.vector.tensor_tensor(z[:, :], y[:, :],
                                        invm_sb[:, l * D:(l + 1) * D], Alu.mult)
                if has_cb[l]:
                    z2 = eppool.tile([128, 128], dt.float32, tag="z2")
                    nc.vector.tensor_tensor(z2[:, :], z[:, :], aux_ap(l, 0), Alu.add)
                    z = z2
                nc.vector.tensor_reduce(
                    s0[:, :], z[:, :], mybir.AxisListType.X, Alu.add)
                nmu = eppool.tile([128, 1], dt.float32, tag="nmu")
                nc.vector.tensor_scalar(nmu[:, :], s0[:, :], -1.0 / D, None, Alu.mult)
                cen = eppool.tile([128, 128], dt.float32, tag="cen")
                nc.scalar.activation(cen[:, :], z[:, :], Act.Identity, bias=nmu[:, :])
                sqs = eppool.tile([128, 128], dt.float32, tag="sqs")
                sq = eppool.tile([128, 1], dt.float32, tag="sq")
                nc.scalar.activation(sqs[:, :], cen[:, :], Act.Square,
                                     accum_out=sq[:, :])
                sd = eppool.tile([128, 1], dt.float32, tag="sd")
                nc.scalar.activation(sd[:, :], sq[:, :], Act.Sqrt,
                                     scale=1.0 / D, bias=eps_sb[:, :])
                rstd = eppool.tile([128, 1], dt.float32, tag="rstd")
                nc.vector.reciprocal(rstd[:, :], sd[:, :])
                h_sb = eppool.tile([128, 128], dt.bfloat16, tag="h_sb")
                if has_lnw[l] or has_lnb[l]:
                    g1 = eppool.tile([128, 128], dt.float32, tag="g1")
                    nc.scalar.activation(g1[:, :], cen[:, :], Act.Copy,
                                         scale=rstd[:, :])
                    g2 = eppool.tile([128, 128], dt.float32, tag="g2")
                    nc.vector.tensor_tensor(g2[:, :], g1[:, :], aux_ap(l, 1), Alu.mult)
                    g3 = eppool.tile([128, 128], dt.float32, tag="g3")
                    nc.vector.tensor_tensor(g3[:, :], g2[:, :], aux_ap(l, 2), Alu.add)
                    nc.vector.tensor_scalar(h_sb[:, :], g3[:, :], 0.0, None, Alu.max)
                else:
                    nc.scalar.activation(h_sb[:, :], cen[:, :], Act.Relu,
                                         scale=rstd[:, :])
                make_hT(h_sb, b)

            def transform(l, b):
                """fl/fr = hT[b] @ (Wlf, Wrf) -> DRAM tables for layer l."""
                hTb = hTt[b][:, :]
                pf = psC.tile([128, 256], dt.float32, tag="pf")
                nc.tensor.matmul(pf[:, 0:128], lhsT=hTb,
                                 rhs=wlf_sb[:, l * D:(l + 1) * D],
                                 start=True, stop=True)
                nc.tensor.matmul(pf[:, 128:256], lhsT=hTb,
                                 rhs=wrf_sb[:, l * D:(l + 1) * D],
                                 start=True, stop=True)
                flfr = eppool.tile([128, 256], dt.bfloat16, tag="flfr")
                if has_blf[l]:
                    t1 = eppool.tile([128, 128], dt.float32, tag="flb")
                    nc.vector.tensor_tensor(t1[:, :], pf[:, 0:128], aux_ap(l, 3), Alu.add)
                    nc.scalar.copy(flfr[:, 0:128], t1[:, :])
                else:
                    nc.scalar.copy(flfr[:, 0:128], pf[:, 0:128])
                if has_brf[l]:
                    t2 = eppool.tile([128, 128], dt.float32, tag="frb")
                    nc.vector.tensor_tensor(t2[:, :], pf[:, 128:256], aux_ap(l, 4), Alu.add)
                    nc.vector.tensor_copy(flfr[:, 128:256], t2[:, :])
                else:
                    nc.scalar.copy(flfr[:, 128:256], pf[:, 128:256])
                nc.sync.dma_start(fl_own[l][b * 128:(b + 1) * 128, :], flfr[:, 0:128])
                nc.sync.dma_start(fr_loc[l][b * 128:(b + 1) * 128, :], flfr[:, 128:256])

            def logits_out(b):
                pl = psB.tile([128, 1], dt.float32, tag="ps1")
                nc.tensor.matmul(pl[:, :], lhsT=hTt[b][:, :],
                                 rhs=wout_sb[:, :], start=True, stop=True)
                if has_b_out:
                    nc.vector.tensor_scalar(out_sb[:, b:b + 1], pl[:, :],
                                            wp["b_out"], None, Alu.add)
                else:
                    nc.vector.tensor_copy(out_sb[:, b:b + 1], pl[:, :])

            def after_hT(l_next, b):
                if l_next < L:
                    transform(l_next, b)
                else:
                    logits_out(b)

            # ---- layer 0: h0 = relu(x @ W_in + b_in) ----
            for b in range(NB):
                xb = xpool.tile([128, KC * 128], dt.bfloat16, tag="xb")
                nc.sync.dma_start(xb[:, :], xT[b, :, :])
                p0 = psA.tile([128, 128], dt.float32, tag="acc128")
                for kc in range(KC):
                    nc.tensor.matmul(
                        p0[:, :], lhsT=xb[:, kc * 128:(kc + 1) * 128],
                        rhs=w_in_sb[:, kc * D:(kc + 1) * D],
                        start=(kc == 0), stop=(kc == KC - 1))
                h_sb = eppool.tile([128, 128], dt.bfloat16, tag="h_sb")
                if has_b_in:
                    hb = eppool.tile([128, 128], dt.float32, tag="hb0")
                    nc.vector.tensor_tensor(hb[:, :], p0[:, :], aux_ap(0, 5), Alu.add)
                    nc.scalar.activation(h_sb[:, :], hb[:, :], Act.Relu)
                else:
                    nc.scalar.activation(h_sb[:, :], p0[:, :], Act.Relu)
                make_hT(h_sb, b)
                after_hT(0, b)

            # ---- GAT layers ----
            for l in range(L):
                K = lay[l]["K"]
                assert 0 < K < D, f"degenerate attention sign split K={K}"
                # phase B: AllGather fl (two halves so the first can overlap
                # the previous layer's edge phase; fl_full is ordered
                # [half, core, half-slab] and gather indices account for it)
                NH = int(os.environ.get("GATV2_HALVES", "2"))
                H2 = NPC_PAD // NH
                for h in range(NH):
                    if NC > 1 and not os.environ.get("GATV2_NOCC"):
                        nc.gpsimd.collective_compute(
                            "AllGather", Alu.bypass,
                            replica_groups=[list(range(NC))],
                            ins=[fl_own[l][h * H2:(h + 1) * H2, :]],
                            outs=[fl_full[l][h * NC * H2:(h + 1) * NC * H2, :]],
                        )
                    else:
                        # 1-core fallback / GATV2_NOCC timing probe: copy the
                        # own slab only (timing-equivalent, results invalid
                        # for NC>1)
                        nc.sync.dma_start(
                            fl_full[l][h * NC * H2:h * NC * H2 + H2, :],
                            fl_own[l][h * H2:(h + 1) * H2, :])
                # phase C: edge aggregation, two blocks per gather group.
                # Slot-aligned layout: partition p's edges all target node p
                # of the block, so fr is a direct block load (broadcast-added
                # over tiles), the softmax matrix is diag(pex), and the
                # denominator is a row-reduce of pex.
                for b0 in range(0, NB, 2):
                    blocks = [b for b in (b0, b0 + 1) if b < NB]
                    Tp = int(sum(T[b] for b in blocks))
                    nidx = Tp * 128
                    o16 = int(offs[b0]) // 16
                    o128 = int(offs[b0]) // 128
                    flg = gpoolG.tile([128, Tp * 128], dt.bfloat16, tag="flg")
                    nc.gpsimd.dma_gather(
                        flg[:, :].rearrange("p (t d) -> p t d", d=128),
                        fl_full[l][:, :], sidx_all[:, o16:o16 + nidx // 16],
                        nidx, nidx, D, single_packet=False)
                    th = gpoolC.tile([128, Tp * 128], dt.bfloat16, tag="th")
                    th3 = th[:, :].rearrange("p (t d) -> p t d", d=128)
                    toff = 0
                    frBs = []
                    for b in blocks:
                        Tb = int(T[b])
                        frB = epool.tile([128, 128], dt.bfloat16, tag="frB")
                        nc.sync.dma_start(frB[:, :],
                                          fr_loc[l][b * 128:(b + 1) * 128, :])
                        frBs.append(frB)
                        nc.vector.tensor_tensor(
                            th3[:, toff:toff + Tb, :],
                            flg[:, :].rearrange("p (t d) -> p t d", d=128)
                            [:, toff:toff + Tb, :],
                            frB[:, :].rearrange("p (o d) -> p o d", o=1)
                            .to_broadcast((128, Tb, D)),
                            Alu.add)
                        toff += Tb
                    sc = gpoolC.tile([128, Tp * 128], dt.bfloat16, tag="sc")
                    sc3 = sc[:, :].rearrange("p (t d) -> p t d", d=128)
                    nc.scalar.activation(sc3[:, :, 0:K], th3[:, :, 0:K], Act.Abs,
                                         scale=2.0 / 3.0)
                    nc.scalar.activation(sc3[:, :, K:D], th3[:, :, K:D], Act.Abs,
                                         scale=2.0 / 3.0)
                    u = gpoolC.tile([128, Tp * 128], dt.bfloat16, tag="u")
                    u3 = u[:, :].rearrange("p (t d) -> p t d", d=128)
                    nc.vector.tensor_tensor(u3[:, :, 0:K], th3[:, :, 0:K],
                                            sc3[:, :, 0:K], Alu.add)
                    nc.vector.tensor_tensor(u3[:, :, K:D], th3[:, :, K:D],
                                            sc3[:, :, K:D], Alu.subtract)
                    logit = eppool.tile([128, 64], dt.float32, tag="logit")
                    nc.vector.tensor_reduce(
                        logit[:, 0:Tp], u3, mybir.AxisListType.X, Alu.add)
                    lgm = eppool.tile([128, 64], dt.float32, tag="lgm")
                    nc.vector.tensor_tensor(lgm[:, 0:Tp], logit[:, 0:Tp],
                                            mask_all[:, o128:o128 + Tp], Alu.add)
                    pex = eppool.tile([128, 64], dt.float32, tag="pex")
                    nc.scalar.activation(pex[:, 0:Tp], lgm[:, 0:Tp], Act.Exp)
                    toff = 0
                    for b in blocks:
                        Tb = int(T[b])
                        pnum = psA.tile([128, 128], dt.float32, tag="acc128")
                        den = eppool.tile([128, 1], dt.float32, tag="den0")
                        nc.vector.tensor_reduce(
                            den[:, :], pex[:, toff:toff + Tb],
                            mybir.AxisListType.X, Alu.add)
                        for t in range(Tb):
                            tg = toff + t
                            FL = flg[:, tg * 128:(tg + 1) * 128]
                            Sp = epool.tile([128, 128], dt.bfloat16, tag="Sp")
                            nc.vector.tensor_scalar(
                                Sp[:, :], ident_sb[:, :],
                                pex[:, tg:tg + 1], None, Alu.mult)
                            nc.tensor.matmul(pnum[:, :], lhsT=Sp[:, :], rhs=FL,
                                             start=(t == 0), stop=(t == Tb - 1))
                        toff += Tb
                        _epilogue(l, b, pnum, den)
                        after_hT(l + 1, b)
                # ---- end of phase C ----

            # ---- output ----
            nc.sync.dma_start(out_d[:, :], out_sb[:, :])

    nc.compile()
    return nc


def _make_in_maps(cfg, wp, x, src_g, fr_g, dl_g, node_perm):
    bf16 = ml_dtypes.bfloat16
    D, L, NC, NPC, NPC_PAD = cfg.D, cfg.L, cfg.NC, cfg.NPC, cfg.NPC_PAD
    lay = wp["layers"]

    wlf = np.stack([l["wlf"] for l in lay])                       # [L,128,D]
    wrf = np.stack([l["wrf"] for l in lay])
    invm = np.stack([np.tile((1.0 / l["m"].astype(np.float64)).astype(np.float32),
                             (128, 1)) for l in lay])             # [L,128,D]
    ident = np.eye(128, dtype=bf16)
    aux = np.zeros((L, 6, 128, D), dtype=np.float32)
    for l in range(L):
        aux[l, 0] = np.tile(lay[l]["cb"], (128, 1))
        aux[l, 1] = np.tile(lay[l]["lnw"], (128, 1))
        aux[l, 2] = np.tile(lay[l]["lnb"], (128, 1))
        aux[l, 3] = np.tile(lay[l]["blf"], (128, 1))
        aux[l, 4] = np.tile(lay[l]["brf"], (128, 1))
        aux[l, 5] = np.tile(wp["b_in"], (128, 1))

    common = dict(
        w_in=np.ascontiguousarray(wp["w_in"]),
        wlf=np.ascontiguousarray(wlf), wrf=np.ascontiguousarray(wrf),
        invm=np.ascontiguousarray(invm),
        wout=np.ascontiguousarray(wp["w_out"]),
        ident=np.ascontiguousarray(ident),
    )
    xs_all = np.zeros((cfg.NPAD, cfg.D_IN), dtype=np.float32)
    xs_all[node_perm] = x
    in_maps = []
    for c in range(NC):
        xs = xs_all[c * NPC_PAD:(c + 1) * NPC_PAD]
        m = dict(common)
        xtd = xs.astype(bf16).reshape(cfg.NB, 128, cfg.KC, 128) \
            .transpose(0, 3, 2, 1).reshape(cfg.NB, 128, cfg.KC * 128)
        m["xT"] = np.ascontiguousarray(xtd)
        m["src_gidx"] = np.ascontiguousarray(src_g[c])
        m["mask"] = np.ascontiguousarray(dl_g[c])
        in_maps.append(m)
    # drop aux if program doesn't use it (detect via any nontrivial flag)
    need_aux = (np.any(wp["b_in"]) or any(np.any(l["blf"]) or np.any(l["brf"])
                or np.any(l["cb"]) or np.any(l["lnb"]) or not np.allclose(l["lnw"], 1.0)
                for l in lay))
    if need_aux:
        for m in in_maps:
            m["aux"] = aux
    return in_maps


def _run_pjrt(nc, in_maps, n_cores, time_iters=0):
    """Mirror of bass2jax.run_bass_via_pjrt's multi-core path, with the jitted
    executable kept so repeated executions can be timed (inputs staged on
    device; only the tiny donated output-zero buffers re-transferred)."""
    import time as _time

    import jax
    import concourse.mybir as mybir
    from concourse import bass2jax
    from jax.experimental.shard_map import shard_map
    from jax.sharding import Mesh, NamedSharding, PartitionSpec

    bass2jax.install_neuronx_cc_hook()
    partition_name = nc.partition_id_tensor.name if nc.partition_id_tensor else None
    in_names, out_names, out_avals, zero_outs = [], [], [], []
    for alloc in nc.m.functions[0].allocations:
        if not isinstance(alloc, mybir.MemoryLocationSet):
            continue
        name = alloc.memorylocations[0].name
        if alloc.kind == "ExternalInput":
            if name != partition_name:
                in_names.append(name)
        elif alloc.kind == "ExternalOutput":
            shape = tuple(alloc.tensor_shape)
            dtype = mybir.dt.np(alloc.dtype)
            out_names.append(name)
            out_avals.append(jax.core.ShapedArray(shape, dtype))
            zero_outs.append(np.zeros(shape, dtype))
    n_params = len(in_names)
    n_outs = len(out_avals)
    all_names = in_names + out_names + ([partition_name] if partition_name else [])

    def _body(*args):
        operands = list(args)
        if partition_name is not None:
            operands.append(bass2jax.partition_id_tensor())
        outs = bass2jax._bass_exec_p.bind(
            *operands, out_avals=tuple(out_avals), in_names=tuple(all_names),
            out_names=tuple(out_names), lowering_input_output_aliases=(),
            sim_require_finite=True, sim_require_nnan=True, nc=nc)
        return tuple(outs)

    devices = jax.devices()[:n_cores]
    mesh = Mesh(np.asarray(devices), ("core",))
    donate = tuple(range(n_params, n_params + n_outs))
    sharded = jax.jit(
        shard_map(_body, mesh=mesh,
                  in_specs=(PartitionSpec("core"),) * (n_params + n_outs),
                  out_specs=(PartitionSpec("core"),) * n_outs, check_rep=False),
        donate_argnums=donate, keep_unused=True)

    concat_in = [
        np.concatenate([np.asarray(in_maps[c][nm]) for c in range(n_cores)], axis=0)
        for nm in in_names]
    sh = NamedSharding(mesh, PartitionSpec("core"))
    staged = [jax.device_put(a, sh) for a in concat_in]

    def zeros():
        return [np.zeros((n_cores * z.shape[0], *z.shape[1:]), z.dtype)
                for z in zero_outs]

    out_arrs = sharded(*staged, *zeros())
    out_np = [np.asarray(a) for a in out_arrs]
    results = [
        {nm: out_np[i].reshape(n_cores, *out_avals[i].shape)[c]
         for i, nm in enumerate(out_names)}
        for c in range(n_cores)]

    best = None
    if time_iters:
        # batched-dispatch slope: wall(k2 calls) - wall(k1 calls) amortizes
        # the RPC round-trip; successive executions serialize on-device.
        def run_k(k):
            zss = [zeros() for _ in range(k)]
            t0 = _time.perf_counter()
            o = None
            for zs in zss:
                o = sharded(*staged, *zs)
            jax.block_until_ready(o)
            return _time.perf_counter() - t0

        run_k(2)  # warm
        k1, k2 = 2, 2 + time_iters
        t1 = min(run_k(k1) for _ in range(4))
        t2 = min(run_k(k2) for _ in range(4))
        best = (t2 - t1) / (k2 - k1)
    return results, best


_cache = {}
last_results = None
last_exec_s = None
last_floor_s = None


def kernel(x, edge_index, W_in, b_in, Wl, bl, Wr, br, att, conv_bias, ln_w, ln_b,
           W_out, b_out):
    global last_results
    x = np.asarray(x); edge_index = np.asarray(edge_index)
    cfg = Cfg(N=x.shape[0], E=edge_index.shape[1], D_IN=x.shape[1],
              D=np.asarray(Wl).shape[2], L=np.asarray(Wl).shape[0], NC=8)

    T, offs, EPAD, src_g, fr_g, dl_g, node_perm = _prep_edges(cfg, edge_index)
    wp = _prep_weights(cfg, W_in, b_in, Wl, bl, Wr, br, att, conv_bias,
                       ln_w, ln_b, W_out, b_out)

    key = (cfg.N, cfg.E, cfg.D_IN, cfg.D, cfg.L, tuple(T),
           os.environ.get("GATV2_NOCC", ""), os.environ.get("GATV2_HALVES", "2"))
    if key not in _cache:
        _cache[key] = _build_program(cfg, wp, T, offs, EPAD)
    nc = _cache[key]

    in_maps = _make_in_maps(cfg, wp, x, src_g, fr_g, dl_g, node_perm)

    global last_exec_s, last_floor_s
    time_iters = int(os.environ.get("GATV2_TIME", "0"))
    if time_iters:
        results, last_exec_s = _run_pjrt(nc, in_maps, cfg.NC, time_iters=time_iters)
        fkey = "floor"
        if fkey not in _cache:
            _cache[fkey] = _floor_program(cfg.NC)
        fin = [{"fa": np.zeros((128, 20), np.float32)} for _ in range(cfg.NC)]
        _, last_floor_s = _run_pjrt(_cache[fkey], fin, cfg.NC, time_iters=time_iters)
    else:
        from concourse import bass_utils
        res = bass_utils.run_bass_kernel_spmd(
            nc, in_maps, core_ids=list(range(cfg.NC)))
        last_results = res
        results = res.results

    full = np.empty(cfg.NPAD, dtype=np.float32)
    for c in range(cfg.NC):
        o = results[c]["out"]                        # [128, NB] f32
        full[c * cfg.NPC_PAD:(c + 1) * cfg.NPC_PAD] = o.T.reshape(-1)
    return full[node_perm].reshape(cfg.N, 1)


def _floor_program(NC):
    """Trivial SPMD program to measure the dispatch/RPC floor."""
    import concourse.bacc as bacc
    import concourse.mybir as mybir
    import concourse.tile as tile
    dt = mybir.dt
    nc = bacc.Bacc("TRN2", target_bir_lowering=False, debug=False,
                   enable_asserts=False, num_devices=NC)
    a = nc.dram_tensor("fa", [128, 20], dt.float32, kind="ExternalInput")
    o = nc.dram_tensor("out", [128, 20], dt.float32, kind="ExternalOutput")
    with tile.TileContext(nc) as tc:
        with tc.tile_pool(name="p", bufs=1) as pool:
            t = pool.tile([128, 20], dt.float32, tag="t")
            nc.sync.dma_start(t[:, :], a[:, :])
            nc.sync.dma_start(o[:, :], t[:, :])
    nc.compile()
    return nc
